# revision 28
# baseline (speedup 1.0000x reference)
"""AdaptiveSpan attention (full span) on 8 TRN2 NeuronCores.

Reference computes, per (b, h) pair:
    s = q @ k.T                     [S, S]
    w = softmax(s * c),  c = SCALE / temperature
    out = w @ v                     [S, D]
and returns (out [B,S,DIMS], w [B,H,S,S]).

Sharding: the 32 (b, h) pairs are split 4-per-core across 8 cores; no
cross-core communication.

Per-core device program, per pair (single-exp design):
  Phase B (k-major):
    sT[k,q] tiles = matmul(lhsT=kT, rhs=qT)   (d contraction zero-padded
    to K=128 — K=64 matmuls never warm the PE clock gate)
    eT = exp(sT) -> bf16, kept in SBUF for the whole pair
    outT[d,q] + a row of softmax denominators l[q] accumulated via
    matmul with v augmented with a ones column; DMA outT (host divides
    by l and transposes — tiny).
    l row is PE-transposed into columns; VectorE reciprocal -> r[q].
  Phase W (weights output):
    e[q,k] tiles obtained by PE-transposing eT 128x128 blocks (bf16,
    ~107 ns each, pipelined), normalized by r via VectorE tensor_scalar
    (bf16 PSUM -> f32 SBUF), and DMA'd out as contiguous 1 MiB blocks.

Only ONE exp pass over the S*S scores per pair (ScalarE is the scarce
engine). All matmul operands are bf16 (fp32 matmuls cost two half-speed
passes; bf16 scores cost ~0.5% relative error, well inside tolerance).
The softmax scale c is folded into q on the host, so the device program
is independent of span_scale.
"""

import numpy as np
import ml_dtypes

HEAD = 16
DIMS = 1024
HEAD_DIM = 64
MAX_DIST = 2048
TEMP_SCALE = 0.01
SHARPEN = True
SCALE = HEAD_DIM ** (-0.25)

B = 2
S = 2048
N_CORES = 8
PAIRS = B * HEAD          # 32
PPC = PAIRS // N_CORES    # 4 pairs per core

_compiled_nc = None


def _build():
    from contextlib import ExitStack
    import concourse.bass as bass
    import concourse.tile as tile
    from concourse import bacc, mybir

    f32 = mybir.dt.float32
    bf16 = mybir.dt.bfloat16
    Exp = mybir.ActivationFunctionType.Exp

    nc = bacc.Bacc("TRN2", target_bir_lowering=False, debug=False,
                   enable_asserts=False, num_devices=N_CORES)

    qT_d = nc.dram_tensor("qT", [PPC, 128, S], bf16, kind="ExternalInput")
    kT_d = nc.dram_tensor("kT", [PPC, 128, S], bf16, kind="ExternalInput")
    va_d = nc.dram_tensor("vaug", [PPC, S, 128], bf16, kind="ExternalInput")
    id_d = nc.dram_tensor("ident", [128, 128], bf16, kind="ExternalInput")
    idf_d = nc.dram_tensor("identf", [128, 128], f32, kind="ExternalInput")
    # w leaves the chip as bf16 (halves the dominant DMA stream); the host
    # upcasts to f32. Costs ~0.4% relative error on top of the bf16 scores.
    w_d = nc.dram_tensor("w", [PPC, S, S], bf16, kind="ExternalOutput")
    ot_d = nc.dram_tensor("outT", [PPC, HEAD_DIM + 1, S], f32, kind="ExternalOutput")

    with tile.TileContext(nc) as tc, ExitStack() as ctx:
        inp = ctx.enter_context(tc.tile_pool(name="inp", bufs=2))
        epool = ctx.enter_context(tc.tile_pool(name="eT", bufs=2))
        wpool = ctx.enter_context(tc.tile_pool(name="wtile", bufs=3))
        small = ctx.enter_context(tc.tile_pool(name="small", bufs=8))
        otpool = ctx.enter_context(tc.tile_pool(name="ot", bufs=4))
        idpool = ctx.enter_context(tc.tile_pool(name="ident", bufs=1))
        spsum = ctx.enter_context(tc.tile_pool(name="s", bufs=2, space="PSUM"))
        opsum = ctx.enter_context(tc.tile_pool(name="o", bufs=2, space="PSUM"))
        tpsum = ctx.enter_context(tc.tile_pool(name="trw", bufs=2, space="PSUM"))

        idt = idpool.tile([128, 128], bf16, tag="ident")
        idf = idpool.tile([128, 128], f32, tag="identf")

        for p in range(PPC):
            # split the input loads so the first matmuls only wait on the
            # first 512-column chunk, not the whole pair
            qt = inp.tile([128, S], bf16, tag="qt")
            for c in range(4):
                nc.sync.dma_start(qt[:, c * 512:(c + 1) * 512],
                                  qT_d.ap()[p][:, c * 512:(c + 1) * 512])
            kt = inp.tile([128, S], bf16, tag="kt")
            for c in range(4):
                nc.sync.dma_start(kt[:, c * 512:(c + 1) * 512],
                                  kT_d.ap()[p][:, c * 512:(c + 1) * 512])
            vt = inp.tile([128, 16 * 128], bf16, tag="vt")
            nc.sync.dma_start(
                vt[:].rearrange("p (c m) -> p c m", c=16),
                va_d.ap()[p].rearrange("(c p) m -> p c m", p=128),
            )
            if p == 0:
                # identities are first needed at the l-transposes; load them
                # after the critical pair-0 q/k chunks
                nc.sync.dma_start(idt[:], id_d.ap()[:])
                nc.sync.dma_start(idf[:], idf_d.ap()[:])

            # eT holds exp(scores^T) for the whole pair: col = kc*S + q
            eT = epool.tile([128, 16 * S], bf16, tag="eT")

            # ---- Phase B: attention output (k on partitions) ----
            # QK matmuls have only a d=64 contraction; rows 64..127 of qt/kt
            # hold a duplicate of rows 0..63, so two consecutive k-chunks run
            # CONCURRENTLY in the upper/lower halves of the PE array
            # (tile_position row-tiling).
            eT3 = eT[:].rearrange("p (c q) -> p c q", c=16)
            rcp_all = small.tile([128, 16], f32, tag="rcp")
            lbfs = []
            for qb in range(4):
                oo = opsum.tile([128, 512], f32, tag="o")
                for kp in range(8):
                    kc = 2 * kp
                    sT = spsum.tile([128, 1024], f32, tag="s")
                    nc.tensor.matmul(
                        sT[:, 0:512],
                        kt[0:64, kc * 128:(kc + 1) * 128],
                        qt[0:64, qb * 512:(qb + 1) * 512],
                        start=True, stop=True, tile_position=(0, 0),
                    )
                    nc.tensor.matmul(
                        sT[:, 512:1024],
                        kt[64:128, (kc + 1) * 128:(kc + 2) * 128],
                        qt[64:128, qb * 512:(qb + 1) * 512],
                        start=True, stop=True, tile_position=(64, 0),
                    )
                    ecur = eT3[:, kc:kc + 2, qb * 512:(qb + 1) * 512]
                    nc.scalar.activation(ecur, sT[:], Exp)
                    nc.tensor.matmul(oo[:], vt[:, kc * 128:(kc + 1) * 128],
                                     eT[:, kc * S + qb * 512: kc * S + (qb + 1) * 512],
                                     start=(kc == 0), stop=False)
                    nc.tensor.matmul(oo[:], vt[:, (kc + 1) * 128:(kc + 2) * 128],
                                     eT[:, (kc + 1) * S + qb * 512: (kc + 1) * S + (qb + 1) * 512],
                                     start=False, stop=(kc == 14))
                ot = otpool.tile([65, 512], f32, tag="ot")
                nc.vector.tensor_copy(ot[:], oo[0:65, :])
                nc.sync.dma_start(ot_d.ap()[p][:, qb * 512:(qb + 1) * 512], ot[:])
                lbfs.append(ot)

            # reciprocals of l for all 16 q-tiles: PE-transpose the ot
            # row blocks into columns (f32), then VectorE reciprocal
            for qi in range(16):
                ltr = opsum.tile([128, 65], f32, tag="o")
                nc.tensor.transpose(
                    ltr[:], lbfs[qi // 4][:, (qi % 4) * 128:(qi % 4 + 1) * 128],
                    idf[0:65, 0:65])
                nc.vector.reciprocal(rcp_all[:, qi:qi + 1], ltr[:, 64:65])

            # ---- Phase W: weights output via PE transposes of eT ----
            # wtile stays bf16 so the normalize runs in the DVE 2x mode;
            # the SWDGE DMA casts bf16 -> f32 on the way to DRAM.
            for qi in range(16):
                wtile = wpool.tile([128, S], bf16, tag="wtile")
                for g in range(2):
                    trw = tpsum.tile([128, 1024], bf16, tag="trw")
                    for j in range(8):
                        kc = g * 8 + j
                        nc.tensor.transpose(
                            trw[:, j * 128:(j + 1) * 128],
                            eT[:, kc * S + qi * 128: kc * S + (qi + 1) * 128],
                            idt[:])
                    nc.vector.tensor_scalar_mul(
                        wtile[:, g * 1024:(g + 1) * 1024], trw[:],
                        rcp_all[:, qi:qi + 1])
                nc.sync.dma_start(w_d.ap()[p, qi * 128:(qi + 1) * 128, :], wtile[:])

    nc.compile()
    return nc


def _get_nc():
    global _compiled_nc
    if _compiled_nc is None:
        _compiled_nc = _build()
    return _compiled_nc


def _prep_inputs(query, key, value, span_scale):
    q = np.asarray(query, dtype=np.float32)
    k = np.asarray(key, dtype=np.float32)
    v = np.asarray(value, dtype=np.float32)
    sm = float(np.asarray(span_scale))

    if SHARPEN:
        temperature = 1.0 + TEMP_SCALE * (1.0 - sm)
    else:
        temperature = 0.5 + TEMP_SCALE * sm
    c = SCALE / temperature

    q4 = q.reshape(B, S, HEAD, HEAD_DIM)
    k4 = k.reshape(B, S, HEAD, HEAD_DIM)
    v4 = v.reshape(B, S, HEAD, HEAD_DIM)

    # rows 64..127 duplicate rows 0..63 so two K=64 matmuls can row-tile
    # into the upper/lower halves of the PE array
    qT = np.empty((PAIRS, 128, S), dtype=ml_dtypes.bfloat16)
    qT[:, :HEAD_DIM, :] = (
        (q4 * np.float32(c)).transpose(0, 2, 3, 1).reshape(PAIRS, HEAD_DIM, S)
    ).astype(ml_dtypes.bfloat16)
    qT[:, HEAD_DIM:, :] = qT[:, :HEAD_DIM, :]
    kT = np.empty((PAIRS, 128, S), dtype=ml_dtypes.bfloat16)
    kT[:, :HEAD_DIM, :] = (
        k4.transpose(0, 2, 3, 1).reshape(PAIRS, HEAD_DIM, S)
    ).astype(ml_dtypes.bfloat16)
    kT[:, HEAD_DIM:, :] = kT[:, :HEAD_DIM, :]
    vaug = np.zeros((PAIRS, S, 128), dtype=ml_dtypes.bfloat16)
    vaug[:, :, :HEAD_DIM] = (
        v4.transpose(0, 2, 1, 3).reshape(PAIRS, S, HEAD_DIM)
    ).astype(ml_dtypes.bfloat16)
    vaug[:, :, HEAD_DIM] = 1.0
    return qT, kT, vaug


def kernel(query, key, value, span_scale):
    from concourse import bass_utils

    qT, kT, vaug = _prep_inputs(query, key, value, span_scale)
    nc = _get_nc()
    identf = np.eye(128, dtype=np.float32)
    ident = identf.astype(ml_dtypes.bfloat16)
    in_maps = [
        {
            "qT": qT[i * PPC:(i + 1) * PPC],
            "kT": kT[i * PPC:(i + 1) * PPC],
            "vaug": vaug[i * PPC:(i + 1) * PPC],
            "ident": ident,
            "identf": identf,
        }
        for i in range(N_CORES)
    ]
    res = bass_utils.run_bass_kernel_spmd(nc, in_maps, core_ids=list(range(N_CORES)))

    w = np.concatenate(
        [res.results[i]["w"].astype(np.float32) for i in range(N_CORES)], axis=0)
    w = w.reshape(B, HEAD, S, S)
    outT = np.concatenate([res.results[i]["outT"] for i in range(N_CORES)], axis=0)
    out = outT[:, :HEAD_DIM, :] / outT[:, HEAD_DIM:HEAD_DIM + 1, :]
    out = out.reshape(B, HEAD, HEAD_DIM, S).transpose(0, 3, 1, 2).reshape(B, S, DIMS)
    return (np.ascontiguousarray(out, dtype=np.float32), w.astype(np.float32))


# revision 29
# speedup vs baseline: 1.1762x; 1.1762x over previous
"""AdaptiveSpan attention (full span) on 8 TRN2 NeuronCores.

Reference computes, per (b, h) pair:
    s = q @ k.T                     [S, S]
    w = softmax(s * c),  c = SCALE / temperature
    out = w @ v                     [S, D]
and returns (out [B,S,DIMS], w [B,H,S,S]).

Sharding: the 32 (b, h) pairs are split 4-per-core across 8 cores; no
cross-core communication.

Per-core device program, per pair (single-exp design):
  Phase B (k-major):
    sT[k,q] tiles = matmul(lhsT=kT, rhs=qT). The d=64 contraction only
    fills half the PE array, so rows 64..127 of qT/kT duplicate rows
    0..63 and two consecutive k-chunks run CONCURRENTLY via
    tile_position row-tiling.
    eT = exp(sT) -> bf16, kept in SBUF for the whole pair.
    outT[d,q] + a row of softmax denominators l[q] accumulated via
    matmul with v augmented with a ones column; DMA outT (host divides
    by l and transposes — tiny).
    l row is PE-transposed into columns; VectorE reciprocal -> r[q].
  Phase W (weights output):
    e[q,k] tiles obtained by PE-transposing eT 128x128 blocks (bf16,
    ~56 ns sustained, pipelined), normalized by r via VectorE
    tensor_scalar (bf16 PSUM -> bf16 SBUF, 2x mode), and DMA'd out as
    contiguous row-blocks; w leaves the chip bf16, host upcasts to f32.

Only ONE exp pass over the S*S scores per pair (ScalarE is the scarce
engine). All matmul operands are bf16 (fp32 matmuls cost two half-speed
passes; bf16 scores/weights cost ~0.5% relative error, well inside
tolerance). The softmax scale c is folded into q on the host, so the
device program is independent of span_scale.
"""

import numpy as np
import ml_dtypes

HEAD = 16
DIMS = 1024
HEAD_DIM = 64
MAX_DIST = 2048
TEMP_SCALE = 0.01
SHARPEN = True
SCALE = HEAD_DIM ** (-0.25)

B = 2
S = 2048
N_CORES = 8
PAIRS = B * HEAD          # 32
PPC = PAIRS // N_CORES    # 4 pairs per core

_compiled_nc = None


def _build():
    from contextlib import ExitStack
    import concourse.bass as bass
    import concourse.tile as tile
    from concourse import bacc, mybir

    f32 = mybir.dt.float32
    bf16 = mybir.dt.bfloat16
    Exp = mybir.ActivationFunctionType.Exp

    nc = bacc.Bacc("TRN2", target_bir_lowering=False, debug=False,
                   enable_asserts=False, num_devices=N_CORES)

    qT_d = nc.dram_tensor("qT", [PPC, 128, S], bf16, kind="ExternalInput")
    kT_d = nc.dram_tensor("kT", [PPC, 128, S], bf16, kind="ExternalInput")
    va_d = nc.dram_tensor("vaug", [PPC, S, 128], bf16, kind="ExternalInput")
    id_d = nc.dram_tensor("ident", [128, 128], bf16, kind="ExternalInput")
    idf_d = nc.dram_tensor("identf", [128, 128], f32, kind="ExternalInput")
    # w leaves the chip as bf16 (halves the dominant DMA stream); the host
    # upcasts to f32. Costs ~0.4% relative error on top of the bf16 scores.
    w_d = nc.dram_tensor("w", [PPC, S, S], bf16, kind="ExternalOutput")
    ot_d = nc.dram_tensor("outT", [PPC, HEAD_DIM + 1, S], f32, kind="ExternalOutput")

    with tile.TileContext(nc) as tc, ExitStack() as ctx:
        inp = ctx.enter_context(tc.tile_pool(name="inp", bufs=2))
        epool = ctx.enter_context(tc.tile_pool(name="eT", bufs=2))
        wpool = ctx.enter_context(tc.tile_pool(name="wtile", bufs=3))
        small = ctx.enter_context(tc.tile_pool(name="small", bufs=8))
        otpool = ctx.enter_context(tc.tile_pool(name="ot", bufs=4))
        idpool = ctx.enter_context(tc.tile_pool(name="ident", bufs=1))
        spsum = ctx.enter_context(tc.tile_pool(name="s", bufs=2, space="PSUM"))
        opsum = ctx.enter_context(tc.tile_pool(name="o", bufs=2, space="PSUM"))
        tpsum = ctx.enter_context(tc.tile_pool(name="trw", bufs=2, space="PSUM"))

        idt = idpool.tile([128, 128], bf16, tag="ident")
        idf = idpool.tile([128, 128], f32, tag="identf")

        for p in range(PPC):
            # split the input loads so the first matmuls only wait on the
            # first 512-column chunk, not the whole pair
            qt = inp.tile([128, S], bf16, tag="qt")
            for c in range(4):
                nc.sync.dma_start(qt[:, c * 512:(c + 1) * 512],
                                  qT_d.ap()[p][:, c * 512:(c + 1) * 512])
            kt = inp.tile([128, S], bf16, tag="kt")
            for c in range(4):
                nc.sync.dma_start(kt[:, c * 512:(c + 1) * 512],
                                  kT_d.ap()[p][:, c * 512:(c + 1) * 512])
            vt = inp.tile([128, 16 * 128], bf16, tag="vt")
            nc.sync.dma_start(
                vt[:].rearrange("p (c m) -> p c m", c=16),
                va_d.ap()[p].rearrange("(c p) m -> p c m", p=128),
            )
            if p == 0:
                # identities are first needed at the l-transposes; load them
                # after the critical pair-0 q/k chunks
                nc.sync.dma_start(idt[:], id_d.ap()[:])
                nc.sync.dma_start(idf[:], idf_d.ap()[:])

            # eT holds exp(scores^T) for the whole pair: col = kc*S + q
            eT = epool.tile([128, 16 * S], bf16, tag="eT")

            # ---- Phase B: attention output (k on partitions) ----
            # QK matmuls have only a d=64 contraction; rows 64..127 of qt/kt
            # hold a duplicate of rows 0..63, so two consecutive k-chunks run
            # CONCURRENTLY in the upper/lower halves of the PE array
            # (tile_position row-tiling).
            eT3 = eT[:].rearrange("p (c q) -> p c q", c=16)
            rcp_all = small.tile([128, 16], f32, tag="rcp")
            lbfs = []
            for qb in range(4):
                oo = opsum.tile([128, 512], f32, tag="o")
                for kp in range(8):
                    kc = 2 * kp
                    sT = spsum.tile([128, 1024], f32, tag="s")
                    nc.tensor.matmul(
                        sT[:, 0:512],
                        kt[0:64, kc * 128:(kc + 1) * 128],
                        qt[0:64, qb * 512:(qb + 1) * 512],
                        start=True, stop=True, tile_position=(0, 0),
                    )
                    nc.tensor.matmul(
                        sT[:, 512:1024],
                        kt[64:128, (kc + 1) * 128:(kc + 2) * 128],
                        qt[64:128, qb * 512:(qb + 1) * 512],
                        start=True, stop=True, tile_position=(64, 0),
                    )
                    ecur = eT3[:, kc:kc + 2, qb * 512:(qb + 1) * 512]
                    nc.scalar.activation(ecur, sT[:], Exp)
                    nc.tensor.matmul(oo[:], vt[:, kc * 128:(kc + 1) * 128],
                                     eT[:, kc * S + qb * 512: kc * S + (qb + 1) * 512],
                                     start=(kc == 0), stop=False)
                    nc.tensor.matmul(oo[:], vt[:, (kc + 1) * 128:(kc + 2) * 128],
                                     eT[:, (kc + 1) * S + qb * 512: (kc + 1) * S + (qb + 1) * 512],
                                     start=False, stop=(kc == 14))
                ot = otpool.tile([65, 512], f32, tag="ot")
                nc.vector.tensor_copy(ot[:], oo[0:65, :])
                nc.sync.dma_start(ot_d.ap()[p][:, qb * 512:(qb + 1) * 512], ot[:])
                lbfs.append(ot)

            # reciprocals of l for all 16 q-tiles: PE-transpose the ot
            # row blocks into columns (f32), then VectorE reciprocal
            for qi in range(16):
                ltr = opsum.tile([128, 65], f32, tag="o")
                nc.tensor.transpose(
                    ltr[:], lbfs[qi // 4][:, (qi % 4) * 128:(qi % 4 + 1) * 128],
                    idf[0:65, 0:65])
                nc.vector.reciprocal(rcp_all[:, qi:qi + 1], ltr[:, 64:65])

            # ---- Phase W: weights output via PE transposes of eT ----
            # wtile stays bf16 so the normalize runs in the DVE 2x mode;
            # the SWDGE DMA casts bf16 -> f32 on the way to DRAM.
            for qi in range(16):
                wtile = wpool.tile([128, S], bf16, tag="wtile")
                for g in range(2):
                    trw = tpsum.tile([128, 1024], bf16, tag="trw")
                    for j in range(8):
                        kc = g * 8 + j
                        nc.tensor.transpose(
                            trw[:, j * 128:(j + 1) * 128],
                            eT[:, kc * S + qi * 128: kc * S + (qi + 1) * 128],
                            idt[:])
                    nc.vector.tensor_scalar_mul(
                        wtile[:, g * 1024:(g + 1) * 1024], trw[:],
                        rcp_all[:, qi:qi + 1])
                nc.sync.dma_start(w_d.ap()[p, qi * 128:(qi + 1) * 128, :], wtile[:])

    nc.compile()
    return nc


def _get_nc():
    global _compiled_nc
    if _compiled_nc is None:
        _compiled_nc = _build()
    return _compiled_nc


def _prep_inputs(query, key, value, span_scale):
    q = np.asarray(query, dtype=np.float32)
    k = np.asarray(key, dtype=np.float32)
    v = np.asarray(value, dtype=np.float32)
    sm = float(np.asarray(span_scale))

    if SHARPEN:
        temperature = 1.0 + TEMP_SCALE * (1.0 - sm)
    else:
        temperature = 0.5 + TEMP_SCALE * sm
    c = SCALE / temperature

    q4 = q.reshape(B, S, HEAD, HEAD_DIM)
    k4 = k.reshape(B, S, HEAD, HEAD_DIM)
    v4 = v.reshape(B, S, HEAD, HEAD_DIM)

    # rows 64..127 duplicate rows 0..63 so two K=64 matmuls can row-tile
    # into the upper/lower halves of the PE array
    qT = np.empty((PAIRS, 128, S), dtype=ml_dtypes.bfloat16)
    qT[:, :HEAD_DIM, :] = (
        (q4 * np.float32(c)).transpose(0, 2, 3, 1).reshape(PAIRS, HEAD_DIM, S)
    ).astype(ml_dtypes.bfloat16)
    qT[:, HEAD_DIM:, :] = qT[:, :HEAD_DIM, :]
    kT = np.empty((PAIRS, 128, S), dtype=ml_dtypes.bfloat16)
    kT[:, :HEAD_DIM, :] = (
        k4.transpose(0, 2, 3, 1).reshape(PAIRS, HEAD_DIM, S)
    ).astype(ml_dtypes.bfloat16)
    kT[:, HEAD_DIM:, :] = kT[:, :HEAD_DIM, :]
    vaug = np.zeros((PAIRS, S, 128), dtype=ml_dtypes.bfloat16)
    vaug[:, :, :HEAD_DIM] = (
        v4.transpose(0, 2, 1, 3).reshape(PAIRS, S, HEAD_DIM)
    ).astype(ml_dtypes.bfloat16)
    vaug[:, :, HEAD_DIM] = 1.0
    return qT, kT, vaug


def kernel(query, key, value, span_scale):
    from concourse import bass_utils

    qT, kT, vaug = _prep_inputs(query, key, value, span_scale)
    nc = _get_nc()
    identf = np.eye(128, dtype=np.float32)
    ident = identf.astype(ml_dtypes.bfloat16)
    in_maps = [
        {
            "qT": qT[i * PPC:(i + 1) * PPC],
            "kT": kT[i * PPC:(i + 1) * PPC],
            "vaug": vaug[i * PPC:(i + 1) * PPC],
            "ident": ident,
            "identf": identf,
        }
        for i in range(N_CORES)
    ]
    res = bass_utils.run_bass_kernel_spmd(nc, in_maps, core_ids=list(range(N_CORES)))

    w = np.concatenate(
        [res.results[i]["w"].astype(np.float32) for i in range(N_CORES)], axis=0)
    w = w.reshape(B, HEAD, S, S)
    outT = np.concatenate([res.results[i]["outT"] for i in range(N_CORES)], axis=0)
    out = outT[:, :HEAD_DIM, :] / outT[:, HEAD_DIM:HEAD_DIM + 1, :]
    out = out.reshape(B, HEAD, HEAD_DIM, S).transpose(0, 3, 1, 2).reshape(B, S, DIMS)
    return (np.ascontiguousarray(out, dtype=np.float32), w.astype(np.float32))
